# revision 54
# baseline (speedup 1.0000x reference)
"""DualGNN (2x [GCN->BN->ReLU]x2 -> mean-pool -> MLP head) on 8 trn2 NeuronCores.

Strategy
-------
Graphs are data-parallel: core k owns graphs [k*Gpc, (k+1)*Gpc) (batch is
sorted -> contiguous node ranges).  Edges are uniformly random over all nodes,
so each layer gathers rows of a replicated node-feature table.

Linear layers commute with aggregation, so W is applied per *node* before
gathering: the gather tables hold (dinv * h) @ W rows (bf16, padded to
256B rows for dma_gather).  The GCN bias b is dropped entirely — BatchNorm's
mean subtraction makes BN(u + b) == BN(u) exactly.  Per core, per layer:
  - table section build: per-128-slot tile: scale by dinv, PE-transpose,
    matmul W, cast bf16, write to this core's part; AllGather -> full table.
  - aggregation: host sorts each core's edges by (psum block-group, src
    window, dst block).  Windows overlap (int16 reach 32768 > wrow), and the
    flexible boundary edges are assigned so that most (w, blk) cells are an
    exact multiple of 128 on every core — near-zero pad columns.  Gathers
    run on all 4 SWDGE queues (queue kept congruent with tile's 8 DMASW sem
    lanes) with a deep gbuf ring; a bf16 one-hot of local dst (batched
    is_equal vs iota) feeds PE matmuls that accumulate each 128-edge chunk
    straight into the block's slice of a PSUM bank-tile.  Self loops open
    each block's accumulation via an identity matmul (start zeroes the whole
    2KB bank, so start/stop are per bank-tile); one copy per bank-tile lands
    u (bf16) in SBUF, then u *= dinv[dst].
  - BatchNorm: per-core sums/sumsq, a tiny per-(branch,layer) AllReduce,
    stats finished on device, fused apply + ReLU.
  - pooling: flipped matmuls (lhsT=u block, rhs=bf16 graph one-hot) build
    hcT[2H, Gpc] in PSUM directly; invcnt scaling, then the head runs fully
    transposed (bias+ReLU fused into one scalar.activation).

The two branches are software-pipelined: each branch's stats/AllReduce/BN/
table/AllGather chain is emitted inside the other branch's aggregation so
the gather queues never idle.  The SPMD program is identical on every core
(cell targets maxed over cores); all per-core variation is in uploaded
index/scale tensors.
"""

import math

import ml_dtypes
import numpy as np

import concourse.bacc as bacc
import concourse.tile as tile
from concourse import mybir
from concourse.bass import AP
from concourse.bass_utils import run_bass_kernel_spmd

F32 = mybir.dt.float32
BF16 = mybir.dt.bfloat16
I16 = mybir.dt.int16
I32 = mybir.dt.int32
AX = mybir.AxisListType
OP = mybir.AluOpType
ACT = mybir.ActivationFunctionType
BF = ml_dtypes.bfloat16

EPS = 1e-5
NCORES = 8
H = 64
F_IN = 7
F_PAD = 8
TROW = 128           # table row width (bf16) = 256B
G_DEFAULT = 1024

WMAX = 32768         # dma_gather int16 index reach
SCALL = 96           # gather-call budget (columns of 128 edges)
OHB = 16             # one-hot build batch (columns)
GXBUFS = 5           # gather ring depth (gbuf tiles in flight)
GPB = 8              # blocks per PSUM bank-tile ([128, 8*H] f32 = 2KB)
AGB = 4              # PSUM bank-tiles for aggregation
GMAX = GPB * AGB     # blocks per aggregation group (PSUM-resident)


def _ap(t, offset, dims):
    base = t[:] if not isinstance(t, AP) else t
    return AP(base.tensor, base.offset + offset, list(map(tuple, dims)))


# ----------------------------------------------------------------------------
# Host-side preprocessing (graph structure only; all model math is on device)
# ----------------------------------------------------------------------------

def _prep_branch(x, ei, batch, G, ncores):
    x = np.asarray(x, np.float32)
    N, Fin = x.shape
    assert Fin == F_IN
    src = np.asarray(ei[0], np.int64)
    dst = np.asarray(ei[1], np.int64)
    batch = np.asarray(batch, np.int64)

    counts = np.bincount(dst, minlength=N)
    deg = counts + 1
    dinv = (1.0 / np.sqrt(deg.astype(np.float64))).astype(np.float32)

    order = np.argsort(dst, kind="stable")
    src_sorted = src[order]
    rowptr = np.zeros(N + 1, np.int64)
    np.cumsum(counts, out=rowptr[1:])

    Gpc = G // ncores
    gb = np.searchsorted(batch, np.arange(0, G + 1, Gpc))

    cores = []
    for k in range(ncores):
        n0, n1 = int(gb[k]), int(gb[k + 1])
        NB = n1 - n0
        perm = np.argsort(-deg[n0:n1], kind="stable")
        node_order = n0 + perm
        cores.append(dict(n0=n0, NB=NB, node_order=node_order))

    nb = max(math.ceil(c["NB"] / 128) for c in cores)
    NBp = nb * 128
    Ppc = NBp + 128  # +1 zero block

    for k, c in enumerate(cores):
        NB = c["NB"]
        # spread each core's pad slots evenly across blocks so per-block edge
        # counts stay proportional across cores (kills tail-block slack in the
        # uniform max-over-cores schedule)
        bnd = np.rint(NB * np.arange(nb + 1) / nb).astype(np.int64)
        slotpos = np.concatenate(
            [b * 128 + np.arange(bnd[b + 1] - bnd[b]) for b in range(nb)]
        ) if NB else np.zeros(0, np.int64)
        c["slotpos"] = slotpos
        no = np.full(NBp, -1, np.int64)
        no[slotpos] = c["node_order"]
        c["node_order_p"] = no
        # per-edge flat arrays in slot space (incl self loop)
        nodes = c["node_order"]
        lens = rowptr[nodes + 1] - rowptr[nodes]
        tot = int(lens.sum())
        cl = np.zeros(NB + 1, np.int64)
        np.cumsum(lens, out=cl[1:])
        r = np.arange(tot) - np.repeat(cl[:-1], lens)
        e_src = src_sorted[np.repeat(rowptr[nodes], lens) + r]
        e_dst = np.repeat(slotpos, lens)
        # self loops are NOT gathered: each core adds its own part rows
        # back locally (they dominate cross-core window-count variance)
        c["e_src"] = e_src
        c["e_dst"] = e_dst
        # per-core helper tensors
        dp = np.zeros(NBp, np.float32)
        dp[slotpos] = dinv[c["node_order"]]
        c["dinvp"] = dp.reshape(nb, 128).T.copy()
        xp = np.zeros((NBp, F_PAD), np.float32)
        xp[slotpos, :F_IN] = x[c["node_order"]]
        c["xperm"] = xp
        oh = np.zeros((128, nb * Gpc), BF)
        g_local = batch[c["node_order"]] - k * Gpc
        oh[slotpos % 128, (slotpos // 128) * Gpc + g_local] = 1.0
        c["pool_oh"] = oh
        cg = np.bincount(batch, minlength=G)[k * Gpc : (k + 1) * Gpc]
        c["invcnt"] = (1.0 / np.maximum(cg.astype(np.float64), 1.0)).astype(
            np.float32
        )[:, None]

    cfg = dict(N=N, nb=nb, NBp=NBp, Ppc=Ppc, Gpc=Gpc)
    return cfg, cores


def _build_schedule(cfgb, cores, rowmap, nwin, wrow, zrow_w):
    """Group-major chunk schedule + per-core idx/dst arrays for a branch.

    Blocks are grouped into PSUM-resident groups of <=GMAX; within a group
    the window sweep accumulates every block's [128, H] tile in PSUM (one
    start via the self-loop identity matmul, one stop at the block's last
    column), so u is written once per group instead of once per (w, blk).
    """
    nb = cfgb["nb"]
    ncores = len(cores)

    # Window assignment with slack: window w's int16 indices reach rows
    # [w*wrow, w*wrow + WMAX), so edges whose row falls in the first
    # WMAX - wrow rows of window w can instead be served by window w-1.
    # Use that freedom to make every core's (w, blk) cell an exact multiple
    # of 128 for w < last (zero pad), pushing remainders into the last
    # window (one ceil per block instead of nwin).
    flex_lo = WMAX - wrow
    per_core_cells = []  # [k] -> {(w, b): srow array}
    counts = np.zeros((ncores, nwin, nb), np.int64)
    mand = np.zeros((ncores, nwin, nb), np.int64)
    opt = np.zeros((ncores, nwin, nb), np.int64)
    core_edges = []
    for k, c in enumerate(cores):
        srow = rowmap[c["e_src"]]
        blk = c["e_dst"] // 128
        w_hi = srow // wrow
        off = srow - w_hi * wrow
        flex = (off < flex_lo) & (w_hi > 0)
        np.add.at(mand[k], (w_hi[~flex], blk[~flex]), 1)
        np.add.at(opt[k], (w_hi[flex], blk[flex]), 1)
        core_edges.append((srow, c["e_dst"], blk, w_hi, flex))

    # choose per-cell column targets t[w, b]
    cnt = np.zeros((nwin, nb), np.int64)
    # carry[k, b]: flex edges of window w+1 not yet claimed are decided when
    # processing w; pending[k, b] counts edges that MUST land in window w
    pend = mand.copy()  # mandatory per (k, w, b)
    take_from_next = np.zeros((ncores, nwin, nb), np.int64)
    for w in range(nwin):
        for b in range(nb):
            if w == nwin - 1:
                need = [pend[k, w, b] + opt[k, w, b] for k in range(ncores)]
                cnt[w, b] = max(math.ceil(n / 128) for n in need)
                continue
            avail = [pend[k, w, b] + opt[k, w + 1, b] for k in range(ncores)]
            lo = max(pend[k, w, b] for k in range(ncores))
            t = max(min(avail) // 128, math.ceil(lo / 128))
            cnt[w, b] = t
            for k in range(ncores):
                extra = min(max(t * 128 - pend[k, w, b], 0), opt[k, w + 1, b])
                take_from_next[k, w, b] = extra
                opt[k, w + 1, b] -= extra
        # flex edges of w+1 not claimed become mandatory for w+1
        if w + 1 < nwin:
            for b in range(nb):
                for k in range(ncores):
                    pend[k, w + 1, b] += opt[k, w + 1, b]
                    opt[k, w + 1, b] = 0

    # materialize per-core per-cell edge lists
    for k in range(ncores):
        srow, edst, blk, w_hi, flex = core_edges[k]
        w_asn = w_hi.copy()
        # assign flex edges: for each (w+1, b), move take_from_next[k, w, b]
        # of its flex edges down to w; group flex edges by (w_hi, blk)
        flex_idx = np.where(flex)[0]
        fkey = (w_hi[flex_idx] * nb + blk[flex_idx])
        forder = flex_idx[np.argsort(fkey * (1 << 20) + srow[flex_idx], kind="stable")]
        fkey_sorted = w_hi[forder] * nb + blk[forder]
        bounds = np.searchsorted(
            fkey_sorted, np.arange(nwin * nb + 1)
        )
        for w in range(nwin - 1):
            for b in range(nb):
                nmove = int(take_from_next[k, w, b])
                if nmove:
                    key = (w + 1) * nb + b
                    sl = forder[bounds[key] : bounds[key] + nmove]
                    # guard against double-claiming: shrink bounds start
                    bounds[key] += nmove
                    w_asn[sl] = w
        np.add.at(counts[k], (w_asn, blk), 1)
        key2 = np.lexsort((srow, blk, w_asn))
        per_core_cells.append((srow[key2], edst[key2], w_asn[key2], blk[key2]))

    for w in range(nwin):
        for b in range(nb):
            assert cnt[w, b] >= math.ceil(
                max(counts[k, w, b] for k in range(ncores)) / 128
            ), (w, b)

    ngroups = math.ceil(nb / GMAX)
    gbnd = np.rint(nb * np.arange(ngroups + 1) / ngroups).astype(np.int64)
    groups = [list(range(int(gbnd[i]), int(gbnd[i + 1]))) for i in range(ngroups)]

    sched = []  # (g, w, blk, ncols)
    for g, blocks in enumerate(groups):
        for w in range(nwin):
            for b in blocks:
                if cnt[w, b] > 0:
                    sched.append((g, w, b, int(cnt[w, b])))
    totcols = int(sum(s[3] for s in sched))

    # PE matmul start=True zeroes the whole 2KB PSUM bank, so start/stop are
    # per BANK-tile: the bank's first identity matmul starts (zeroing all its
    # block slices), its last column stops. colinfo: per column (blk, stop);
    # gident: per group, per block (start, stop) for the identity matmuls.
    colinfo = []
    last_of_block = {}
    col = 0
    for (g, w, b, n) in sched:
        colinfo.extend([b, False] for _ in range(n))
        last_of_block[b] = col + n - 1
        col += n
    gident = []
    for g, blocks in enumerate(groups):
        flags = []
        for t in range(math.ceil(len(blocks) / GPB)):
            bblocks = blocks[t * GPB : (t + 1) * GPB]
            lasts = [last_of_block[b] for b in bblocks if b in last_of_block]
            if lasts:
                colinfo[max(lasts)][1] = True
            for i, b in enumerate(bblocks):
                flags.append(
                    (i == 0, not lasts and i == len(bblocks) - 1)
                )
        gident.append(flags)

    # calls: contiguous columns sharing (g, w), split at SCALL budget
    gcalls = [[] for _ in groups]
    col = 0
    i = 0
    while i < len(sched):
        g, w = sched[i][0], sched[i][1]
        run = 0
        while i < len(sched) and sched[i][0] == g and sched[i][1] == w:
            run += sched[i][3]
            i += 1
        done = 0
        while done < run:
            n = min(SCALL, run - done)
            gcalls[g].append((w, col, n))
            col += n
            done += n
    assert col == totcols

    # per-core arrays
    idx_all = []
    dst_all = []
    for k in range(ncores):
        srow, edst, _, _ = per_core_cells[k]
        starts = np.zeros(nwin * nb + 1, np.int64)
        np.cumsum(counts[k].reshape(-1), out=starts[1:])
        idx_un = np.zeros(totcols * 128, np.int16)
        dst_un = np.zeros(totcols * 128, np.int16)
        col0 = 0
        for (g, w, b, nc_) in sched:
            s0 = int(starts[w * nb + b])
            e = int(counts[k, w, b])
            sl = slice(col0 * 128, col0 * 128 + e)
            idx_un[sl] = (srow[s0 : s0 + e] - w * wrow).astype(np.int16)
            dst_un[sl] = (edst[s0 : s0 + e] - b * 128).astype(np.int16)
            pad = nc_ * 128 - e
            if pad:
                sl2 = slice(col0 * 128 + e, (col0 + nc_) * 128)
                idx_un[sl2] = zrow_w[w]
                dst_un[sl2] = 0
            col0 += nc_
        # wrap idx per call: item i -> [i%16, i//16]
        wrapped = np.zeros((16, totcols * 8), np.int16)
        for calls in gcalls:
            for (w, c0, n) in calls:
                items = idx_un[c0 * 128 : (c0 + n) * 128]
                wrapped[:, c0 * 8 : (c0 + n) * 8] = items.reshape(n * 8, 16).T
        idx_all.append(np.tile(wrapped, (8, 1)))
        dst_all.append(
            np.ascontiguousarray(dst_un.reshape(totcols, 128).T).astype(BF)
        )

    # sanity: windows within int16
    assert wrow <= WMAX
    maxgcols = max(sum(n for (_, _, n) in calls) for calls in gcalls)
    return dict(
        sched=sched, gcalls=gcalls, colinfo=colinfo, gident=gident,
        groups=groups, totcols=totcols, maxgcols=maxgcols,
    ), idx_all, dst_all


def _prep(inputs, G, ncores):
    cfg_c, cores_c = _prep_branch(
        inputs["x_c"], inputs["edge_index_c"], inputs["batch_c"], G, ncores
    )
    cfg_s, cores_s = _prep_branch(
        inputs["x_s"], inputs["edge_index_s"], inputs["batch_s"], G, ncores
    )
    # per-branch tables: branch b's table is [ncores * Ppc_b, TROW]; windows
    # are per-branch (chromo needs 4, solvent 2 instead of 5 over a unified
    # table), which cuts schedule entries and therefore pad indices.
    for cfgb, cores in ((cfg_c, cores_c), (cfg_s, cores_s)):
        Ppc = cfgb["Ppc"]
        Rtot = ncores * Ppc
        nwin = max(1, math.ceil(Rtot / WMAX))
        wrow = math.ceil(Rtot / nwin)
        zrows = [(k + 1) * Ppc - 1 for k in range(ncores)]
        zrow_w = []
        for w in range(nwin):
            lo, hi = w * wrow, min((w + 1) * wrow, Rtot)
            z = [r for r in zrows if lo <= r < hi]
            assert z, f"no zero row in window {w}"
            zrow_w.append(z[0] - lo)
        N = cfgb["N"]
        rm = np.zeros(N, np.int64)
        for k, c in enumerate(cores):
            rm[c["node_order"]] = k * Ppc + c["slotpos"]
        cfgb["Rtot"] = Rtot
        cfgb["nwin"] = nwin
        cfgb["wrow"] = wrow
        cfgb["zrow_w"] = zrow_w
        cfgb["rowmap"] = rm

    sch_c, idx_c, dst_c = _build_schedule(
        cfg_c, cores_c, cfg_c["rowmap"], cfg_c["nwin"], cfg_c["wrow"],
        cfg_c["zrow_w"]
    )
    sch_s, idx_s, dst_s = _build_schedule(
        cfg_s, cores_s, cfg_s["rowmap"], cfg_s["nwin"], cfg_s["wrow"],
        cfg_s["zrow_w"]
    )
    cfg_c.update(sch_c)
    cfg_s.update(sch_s)
    del cfg_c["rowmap"], cfg_s["rowmap"]

    cfg = dict(c=cfg_c, s=cfg_s, G=G, Gpc=G // ncores)

    # GCNConv bias b is dropped entirely: BatchNorm's mean subtraction makes
    # BN(u + b) == BN(u) exactly (shift invariance), so it never affects the
    # output.
    w = {}
    for br in ("c", "s"):
        W0 = np.zeros((F_PAD, H), np.float32)
        W0[:F_IN] = np.asarray(inputs[f"W{br}0"], np.float32)
        w[f"W0_{br}"] = W0
        w[f"W1_{br}"] = np.asarray(inputs[f"W{br}1"], np.float32)
        for li in (0, 1):
            w[f"g{li}_{br}"] = np.asarray(inputs[f"g{br}{li}"], np.float32)[:, None]
            w[f"beta{li}_{br}"] = np.asarray(
                inputs[f"beta{br}{li}"], np.float32
            )[:, None]
    w["Wf1"] = np.asarray(inputs["Wf1"], np.float32)
    w["bf1T"] = np.asarray(inputs["bf1"], np.float32)[:, None]
    w["Wf2"] = np.asarray(inputs["Wf2"], np.float32)
    w["bf2"] = np.asarray(inputs["bf2"], np.float32)[None, :]

    in_maps = []
    for k in range(ncores):
        m = dict(w)
        for br, cores, idxs, dsts in (
            ("c", cores_c, idx_c, dst_c),
            ("s", cores_s, idx_s, dst_s),
        ):
            c = cores[k]
            m[f"xperm_{br}"] = c["xperm"]
            m[f"ell_{br}"] = idxs[k]
            m[f"dst_{br}"] = dsts[k]
            m[f"dinvp_{br}"] = c["dinvp"]
            m[f"pool_{br}"] = c["pool_oh"]
            m[f"invcntT_{br}"] = c["invcnt"].T.copy()
        in_maps.append(m)
    return cfg, in_maps


# ----------------------------------------------------------------------------
# Device program
# ----------------------------------------------------------------------------

def _build(cfg, stop=None):
    if not hasattr(_build, "flow"):
        _build.flow = "pipe"
    if not hasattr(_build, "probe"):
        _build.probe = False
    nc = bacc.Bacc(
        "TRN2", target_bir_lowering=False, debug=False, num_devices=NCORES,
        num_swdge_queues=4,
    )
    Gpc = cfg["Gpc"]
    rg = [list(range(NCORES))]

    inp = {}
    for br in ("c", "s"):
        b = cfg[br]
        nb, totcols, NBp = b["nb"], b["totcols"], b["NBp"]
        inp[f"xperm_{br}"] = nc.dram_tensor(
            f"xperm_{br}", [NBp, F_PAD], F32, kind="ExternalInput"
        )
        inp[f"ell_{br}"] = nc.dram_tensor(
            f"ell_{br}", [128, totcols * 8], I16, kind="ExternalInput"
        )
        inp[f"dst_{br}"] = nc.dram_tensor(
            f"dst_{br}", [128, totcols], BF16, kind="ExternalInput"
        )
        inp[f"dinvp_{br}"] = nc.dram_tensor(
            f"dinvp_{br}", [128, nb], F32, kind="ExternalInput"
        )
        inp[f"pool_{br}"] = nc.dram_tensor(
            f"pool_{br}", [128, nb * Gpc], BF16, kind="ExternalInput"
        )
        inp[f"invcntT_{br}"] = nc.dram_tensor(
            f"invcntT_{br}", [1, Gpc], F32, kind="ExternalInput"
        )
        inp[f"W0_{br}"] = nc.dram_tensor(
            f"W0_{br}", [F_PAD, H], F32, kind="ExternalInput"
        )
        inp[f"W1_{br}"] = nc.dram_tensor(f"W1_{br}", [H, H], F32, kind="ExternalInput")
        for li in (0, 1):
            for nm, shape in (
                (f"g{li}_{br}", [H, 1]), (f"beta{li}_{br}", [H, 1]),
            ):
                inp[nm] = nc.dram_tensor(nm, shape, F32, kind="ExternalInput")
    inp["Wf1"] = nc.dram_tensor("Wf1", [2 * H, H], F32, kind="ExternalInput")
    inp["bf1T"] = nc.dram_tensor("bf1T", [H, 1], F32, kind="ExternalInput")
    inp["Wf2"] = nc.dram_tensor("Wf2", [H, 2], F32, kind="ExternalInput")
    inp["bf2"] = nc.dram_tensor("bf2", [1, 2], F32, kind="ExternalInput")

    out_t = nc.dram_tensor("out", [Gpc, 2], F32, kind="ExternalOutput")

    part = {
        (li, br): nc.dram_tensor(f"part{li}_{br}", [cfg[br]["Ppc"], TROW], BF16)
        for li in (0, 1) for br in ("c", "s")
    }
    table = {
        (li, br): nc.dram_tensor(
            f"table{li}_{br}", [cfg[br]["Rtot"], TROW], BF16, addr_space="Shared"
        )
        for li in (0, 1) for br in ("c", "s")
    }
    st_in = {
        (br, li): nc.dram_tensor(f"st{li}{br}_in", [2, H], F32)
        for li in (0, 1) for br in ("c", "s")
    }
    st_out = {
        (br, li): nc.dram_tensor(f"st{li}{br}_out", [2, H], F32, addr_space="Shared")
        for li in (0, 1) for br in ("c", "s")
    }

    with tile.TileContext(nc, num_cores=NCORES) as tc:
        consts = tc.alloc_tile_pool(name="consts", bufs=1)
        wpool = tc.alloc_tile_pool(name="weights", bufs=1)
        upool = tc.alloc_tile_pool(name="ubuf", bufs=1)
        gpool = tc.alloc_tile_pool(name="gather", bufs=2)
        gxpool = tc.alloc_tile_pool(name="gatherx", bufs=GXBUFS)
        ohpool = tc.alloc_tile_pool(name="onehot", bufs=2)
        spool = tc.alloc_tile_pool(name="small", bufs=4)
        ppool = tc.alloc_tile_pool(name="psum", bufs=1, space="PSUM")
        xpool = tc.alloc_tile_pool(name="xstage", bufs=2)

        from concourse.masks import make_identity

        ident = consts.tile([128, 128], F32)
        make_identity(nc, ident[:])
        ident_bf = consts.tile([128, 128], BF16)
        nc.vector.tensor_copy(out=ident_bf[:], in_=ident[:])
        ones_col = consts.tile([128, 1], F32)
        nc.gpsimd.memset(ones_col[:], 1.0)
        ones_row = consts.tile([1, 128], F32)
        nc.gpsimd.memset(ones_row[:], 1.0)
        eps_t = consts.tile([H, 1], F32)
        nc.gpsimd.memset(eps_t[:], EPS)
        zero_big = consts.tile([128, TROW], BF16)
        nc.gpsimd.memset(zero_big[:], 0.0)
        iota_i = consts.tile([128, 128], I32)
        nc.gpsimd.iota(iota_i[:], pattern=[[1, 128]], base=0, channel_multiplier=0)
        iota_bf = consts.tile([128, 128], BF16)
        nc.vector.tensor_copy(out=iota_bf[:], in_=iota_i[:])

        def load_w(name, shape):
            t = wpool.tile(list(shape), F32, tag=name, name=f"w_{name}")
            nc.sync.dma_start(out=t[:], in_=inp[name].ap())
            return t

        def replicate_row(row_ap, width, tag):
            ps = ppool.tile([128, width], F32, tag="ps_u", bufs=2)
            nc.tensor.matmul(
                out=ps[:], lhsT=ones_row[:], rhs=row_ap, start=True, stop=True
            )
            t = wpool.tile([128, width], F32, tag=tag, name=f"rep_{tag}")
            nc.vector.tensor_copy(out=t[:], in_=ps[:])
            return t

        Wt = {}
        for br in ("c", "s"):
            Wt[br, 0] = load_w(f"W0_{br}", (F_PAD, H))
            Wt[br, 1] = load_w(f"W1_{br}", (H, H))
        Wf1 = load_w("Wf1", (2 * H, H))
        Wf2 = load_w("Wf2", (H, 2))
        gam = {}
        bet = {}
        for br in ("c", "s"):
            for li in (0, 1):
                gam[br, li] = load_w(f"g{li}_{br}", (H, 1))
                bet[br, li] = load_w(f"beta{li}_{br}", (H, 1))
        bf1T = load_w("bf1T", (H, 1))
        bf2row = spool.tile([1, 2], F32, tag="brow2")
        nc.sync.dma_start(out=bf2row[:], in_=inp["bf2"].ap())
        bf2rep = replicate_row(bf2row[:], 2, "bf2rep")
        icrep = {}
        for br in ("c", "s"):
            icrow = spool.tile([1, Gpc], F32, tag="icrow")
            nc.sync.dma_start(out=icrow[:], in_=inp[f"invcntT_{br}"].ap())
            icrep[br] = replicate_row(icrow[:], Gpc, f"icrep_{br}")

        dinvp_t = {}
        for br in ("c", "s"):
            nb = cfg[br]["nb"]
            dinvp_t[br] = wpool.tile(
                [128, nb], F32, name=f"dinvp_t_{br}", tag=f"dinvp_{br}"
            )
            nc.sync.dma_start(out=dinvp_t[br][:], in_=inp[f"dinvp_{br}"].ap())

        # zero each part's pad/zero block (gathered by pad indices); real rows
        # are fully written by build_table, and cols H..TROW are never read
        for li in (0, 1):
            for br in ("c", "s"):
                nc.sync.dma_start(
                    out=_ap(part[li, br].ap(), cfg[br]["NBp"] * TROW,
                            [(TROW, 128), (1, TROW)]),
                    in_=zero_big[:],
                )

        u_t = {}
        acc2_t = {}
        probe_acc = None
        if _build.probe:
            probe_acc = upool.tile([128, 1], BF16, tag="probe_acc")
            nc.gpsimd.memset(probe_acc[:], 0.0)

        # ------------------------------------------------------------------
        # table section build: rows <- (dinv * src_rows) @ W  (bf16, cols 0:64)
        # ------------------------------------------------------------------
        def build_table(br, li):
            b = cfg[br]
            nb = b["nb"]
            for t in range(nb):
                if li == 0:
                    xt = xpool.tile([128, F_PAD], F32, tag="xt")
                    nc.sync.dma_start(
                        out=xt[:],
                        in_=_ap(inp[f"xperm_{br}"].ap(), t * 128 * F_PAD,
                                [(F_PAD, 128), (1, F_PAD)]),
                    )
                    fin = F_PAD
                else:
                    u = u_t[br]
                    xt = xpool.tile([128, H], F32, tag="xt1")
                    nc.vector.tensor_tensor(
                        out=xt[:], in0=u[:, t * H : (t + 1) * H],
                        in1=_ap(dinvp_t[br], t, [(cfg[br]["nb"], 128), (0, H)]),
                        op=OP.mult,
                    )
                    fin = H
                if li == 0:
                    nc.vector.tensor_tensor(
                        out=xt[:], in0=xt[:],
                        in1=_ap(dinvp_t[br], t, [(nb, 128), (0, F_PAD)]),
                        op=OP.mult,
                    )
                zT_ps = ppool.tile([fin, 128], F32, tag="ps_t", bufs=2)
                nc.tensor.transpose(out=zT_ps[:], in_=xt[:], identity=ident[:])
                zT = xpool.tile([fin, 128], F32, tag=f"zT{fin}")
                nc.vector.tensor_copy(out=zT[:], in_=zT_ps[:])
                r_ps = ppool.tile([128, H], F32, tag="ps_u", bufs=2)
                nc.tensor.matmul(
                    out=r_ps[:], lhsT=zT[:], rhs=Wt[br, li][:], start=True, stop=True
                )
                stage = xpool.tile([128, H], BF16, tag="stage")
                nc.vector.tensor_copy(out=stage[:], in_=r_ps[:])
                nc.sync.dma_start(
                    out=_ap(part[li, br].ap(), t * 128 * TROW,
                            [(TROW, 128), (1, H)]),
                    in_=stage[:],
                )

        # ------------------------------------------------------------------
        # aggregation: per PSUM-resident block group, the self-loop identity
        # matmul opens each block's accumulator and the window sweep's one-hot
        # matmuls accumulate into it; one copy-out per bank-tile.
        # ------------------------------------------------------------------
        def aggregate(br, li, gset=None):
            b = cfg[br]
            nb = b["nb"]
            u = u_t.get(br)
            if u is None:
                u = upool.tile([128, nb * H], BF16, tag=f"u_{br}", name=f"u_{br}")
                u_t[br] = u
                if _build.probe:
                    nc.gpsimd.memset(u[:], 0.0)
            colinfo = b["colinfo"]
            if gset is None:
                gset = range(len(b["groups"]))
            probe = _build.probe
            for gi in gset:
                blocks = b["groups"][gi]
                ci = aggregate.ci.get((br, li), 0)
                g0 = blocks[0]
                gn = len(blocks)
                nbank = math.ceil(gn / GPB)
                gc0 = b["gcalls"][gi][0][1]
                gcols = sum(n for (_, _, n) in b["gcalls"][gi])
                it_g = gpool.tile(
                    [128, b["maxgcols"] * 8], I16, tag="itg",
                    name=f"itg_{br}{li}{gi}"
                )
                nc.sync.dma_start(
                    out=it_g[:, : gcols * 8],
                    in_=inp[f"ell_{br}"].ap()[:, gc0 * 8 : (gc0 + gcols) * 8],
                )
                if probe:
                    for (w, col0, ncols) in b["gcalls"][gi]:
                        qi = aggregate.gq
                        aggregate.gq += 1
                        gbuf = gxpool.tile([128, SCALL * TROW], BF16, tag="gbuf")
                        num = ncols * 128
                        nc.gpsimd.dma_gather(
                            out_ap=_ap(gbuf, 0,
                                       [(SCALL * TROW, 128), (TROW, ncols),
                                        (1, TROW)]),
                            in_ap=table[li, br].ap()[w * cfg[br]["wrow"] :, :],
                            idxs_ap=it_g[:, (col0 - gc0) * 8 :
                                         (col0 - gc0 + ncols) * 8],
                            num_idxs=num,
                            num_idxs_reg=num,
                            elem_size=TROW,
                            single_packet=False,
                            queue_num=qi % 4,
                        )
                        nc.vector.tensor_tensor(
                            out=probe_acc[:], in0=probe_acc[:],
                            in1=gbuf[:, :1], op=OP.add,
                        )
                    aggregate.ci[(br, li)] = ci + gcols
                    continue
                dst_g = gpool.tile(
                    [128, b["maxgcols"]], BF16, tag="dstg",
                    name=f"dstg_{br}{li}{gi}"
                )
                nc.sync.dma_start(
                    out=dst_g[:, :gcols],
                    in_=inp[f"dst_{br}"].ap()[:, gc0 : gc0 + gcols],
                )
                zb = [
                    ppool.tile([128, GPB * H], F32, tag=f"zbank{t}", bufs=1,
                               name=f"zb_{br}{li}g{gi}t{t}")
                    for t in range(nbank)
                ]
                # self-loop rows, one bank-tile of blocks at a time
                for t in range(nbank):
                    n_ = min(GPB, gn - t * GPB)
                    own = gpool.tile(
                        [128, GPB * H], BF16, tag="own",
                        name=f"own_{br}{li}{gi}t{t}"
                    )
                    nc.sync.dma_start(
                        out=own[:, : n_ * H],
                        in_=_ap(part[li, br].ap(), (g0 + t * GPB) * 128 * TROW,
                                [(TROW, 128), (128 * TROW, n_), (1, H)]),
                    )
                    for i in range(n_):
                        j = t * GPB + i
                        st, sp = b["gident"][gi][j]
                        nc.tensor.matmul(
                            out=zb[t][:, i * H : (i + 1) * H],
                            lhsT=ident_bf[:],
                            rhs=own[:, i * H : (i + 1) * H],
                            start=st,
                            stop=sp,
                        )
                for (w, col0, ncols) in b["gcalls"][gi]:
                    # queue stays congruent with tile's 8-lane DMASW sem
                    # round-robin (lane = global pool-DMA index % 8), so a
                    # sem lane is never shared across SWDGE queues
                    qi = aggregate.gq
                    aggregate.gq += 1
                    gbuf = gxpool.tile([128, SCALL * TROW], BF16, tag="gbuf")
                    num = ncols * 128
                    nc.gpsimd.dma_gather(
                        out_ap=_ap(gbuf, 0,
                                   [(SCALL * TROW, 128), (TROW, ncols), (1, TROW)]),
                        in_ap=table[li, br].ap()[w * cfg[br]["wrow"] :, :],
                        idxs_ap=it_g[:, (col0 - gc0) * 8 :
                                     (col0 - gc0 + ncols) * 8],
                        num_idxs=num,
                        num_idxs_reg=num,
                        elem_size=TROW,
                        single_packet=False,
                        queue_num=qi % 4,
                    )
                    for b0 in range(0, ncols, OHB):
                        bn = min(OHB, ncols - b0)
                        oh = ohpool.tile([128, OHB * 128], BF16, tag="oh")
                        nc.vector.tensor_tensor(
                            out=_ap(oh, 0, [(OHB * 128, 128), (128, bn), (1, 128)]),
                            in0=_ap(dst_g, col0 - gc0 + b0,
                                    [(b["maxgcols"], 128), (1, bn), (0, 128)]),
                            in1=_ap(iota_bf, 0, [(128, 128), (0, bn), (1, 128)]),
                            op=OP.is_equal,
                        )
                        for j in range(bn):
                            cblk, clast = colinfo[ci]
                            ci += 1
                            jj = cblk - g0
                            nc.tensor.matmul(
                                out=zb[jj // GPB][
                                    :, (jj % GPB) * H : (jj % GPB + 1) * H
                                ],
                                lhsT=oh[:, j * 128 : (j + 1) * 128],
                                rhs=_ap(gbuf, (b0 + j) * TROW,
                                        [(SCALL * TROW, 128), (1, H)]),
                                start=False,
                                stop=clast,
                            )
                # copy group accumulators out to u (one copy per bank-tile)
                for t in range(nbank):
                    n_ = min(GPB, gn - t * GPB)
                    nc.vector.tensor_copy(
                        out=u[:, (g0 + t * GPB) * H : (g0 + t * GPB + n_) * H],
                        in_=zb[t][:, : n_ * H],
                    )
                aggregate.ci[(br, li)] = ci

        def agg_finish(br, li):
            b = cfg[br]
            nb = b["nb"]
            assert aggregate.ci.pop((br, li)) == len(b["colinfo"])
            u = u_t[br]
            # u = u * dinvp
            full = _ap(u, 0, [(nb * H, 128), (H, nb), (1, H)])
            nc.vector.tensor_tensor(
                out=full, in0=full,
                in1=_ap(dinvp_t[br], 0, [(nb, 128), (1, nb), (0, H)]), op=OP.mult,
            )

        aggregate.gq = 0
        aggregate.ci = {}

        def layer_stats(br, li):
            st_in_t = st_in[(br, li)]
            b = cfg[br]
            nb = b["nb"]
            u = u_t[br]
            acc2 = spool.tile([128, H], F32, tag=f"acc2_{br}", name=f"acc2_{br}{li}")
            acc2_t[br] = acc2
            sq = spool.tile([128, H], F32, tag="sq")
            nc.scalar.activation(
                out=sq[:], in_=u[:, 0:H], func=ACT.Square
            )
            nc.vector.tensor_copy(out=acc2[:], in_=sq[:])
            for t in range(1, nb):
                sq = spool.tile([128, H], F32, tag="sq")
                nc.scalar.activation(
                    out=sq[:], in_=u[:, t * H : (t + 1) * H], func=ACT.Square
                )
                nc.vector.tensor_tensor(
                    out=acc2[:], in0=acc2[:], in1=sq[:], op=OP.add
                )
            rsum = spool.tile([128, H], F32, tag="rsum")
            nc.vector.tensor_reduce(
                out=rsum[:], in_=_ap(u, 0, [(nb * H, 128), (1, H), (H, nb)]),
                axis=AX.X, op=OP.add,
            )
            su_ps = ppool.tile([H, 1], F32, tag="ps_t", bufs=2)
            nc.tensor.matmul(
                out=su_ps[:], lhsT=rsum[:], rhs=ones_col[:], start=True, stop=True
            )
            s2_ps = ppool.tile([H, 1], F32, tag="ps_t", bufs=2)
            nc.tensor.matmul(
                out=s2_ps[:], lhsT=acc2[:], rhs=ones_col[:], start=True, stop=True
            )
            su = spool.tile([H, 1], F32, tag="su")
            nc.vector.tensor_copy(out=su[:], in_=su_ps[:])
            s2 = spool.tile([H, 1], F32, tag="s2")
            nc.vector.tensor_copy(out=s2[:], in_=s2_ps[:])
            nc.sync.dma_start(out=st_in_t.ap()[0:1, :], in_=su[:])
            nc.sync.dma_start(out=st_in_t.ap()[1:2, :], in_=s2[:])

        def bn_finish(br, li):
            Ntotal = cfg[br]["N"]
            sts = spool.tile([H, 2], F32, tag="sts")
            nc.sync.dma_start(
                out=sts[:], in_=_ap(st_out[(br, li)].ap(), 0, [(1, H), (H, 2)])
            )
            mu = spool.tile([H, 1], F32, tag="mu")
            nc.vector.tensor_scalar_mul(out=mu[:], in0=sts[:, 0:1], scalar1=1.0 / Ntotal)
            ex2 = spool.tile([H, 1], F32, tag="ex2")
            nc.vector.tensor_scalar_mul(out=ex2[:], in0=sts[:, 1:2], scalar1=1.0 / Ntotal)
            musq = spool.tile([H, 1], F32, tag="musq")
            nc.vector.tensor_tensor(out=musq[:], in0=mu[:], in1=mu[:], op=OP.mult)
            var = spool.tile([H, 1], F32, tag="var")
            nc.vector.tensor_tensor(out=var[:], in0=ex2[:], in1=musq[:], op=OP.subtract)
            std = spool.tile([H, 1], F32, tag="std")
            nc.scalar.activation(out=std[:], in_=var[:], func=ACT.Sqrt, bias=eps_t[:])
            istd = spool.tile([H, 1], F32, tag="istd")
            nc.vector.reciprocal(out=istd[:], in_=std[:])
            sc = spool.tile([H, 1], F32, tag="sc")
            nc.vector.tensor_tensor(out=sc[:], in0=gam[br, li][:], in1=istd[:], op=OP.mult)
            sh = spool.tile([H, 1], F32, tag="sh")
            nc.vector.tensor_tensor(out=sh[:], in0=mu[:], in1=sc[:], op=OP.mult)
            nc.vector.tensor_tensor(
                out=sh[:], in0=bet[br, li][:], in1=sh[:], op=OP.subtract
            )
            reps = []
            for vec, tag in ((sc, "screp"), (sh, "shrep")):
                vr_ps = ppool.tile([1, H], F32, tag="ps_t", bufs=2)
                nc.tensor.transpose(out=vr_ps[:], in_=vec[:], identity=ident[:H, :H])
                vr = spool.tile([1, H], F32, tag="vrow")
                nc.vector.tensor_copy(out=vr[:], in_=vr_ps[:])
                reps.append(replicate_row(vr[:], H, f"{tag}_{br}{li}"))
            return reps

        def bn_apply(br, screp, shrep):
            b = cfg[br]
            nb = b["nb"]
            u = u_t[br]
            full = _ap(u, 0, [(nb * H, 128), (H, nb), (1, H)])
            nc.vector.tensor_tensor(
                out=full, in0=full,
                in1=_ap(screp, 0, [(H, 128), (0, nb), (1, H)]), op=OP.mult,
            )
            nc.vector.tensor_tensor(
                out=full, in0=full,
                in1=_ap(shrep, 0, [(H, 128), (0, nb), (1, H)]), op=OP.add,
            )
            flat = u[:, : nb * H]
            nc.scalar.activation(out=flat, in_=flat, func=ACT.Relu)

        # pooling: flipped matmul gives hcT[2H, Gpc] directly; the solvent
        # one-hot is prefetched, the chromo one streams in bf16 via a ring
        hcT = spool.tile([2 * H, Gpc], F32, tag="hcT")

        def pool_branch(br):
            b = cfg[br]
            nb = b["nb"]
            u = u_t[br]
            row0 = 0 if br == "c" else H
            ps = ppool.tile([H, Gpc], F32, tag="ps_u", bufs=2,
                            name=f"poolps_{br}")
            for t in range(nb):
                poh = gxpool.tile([128, Gpc], BF16, tag="pohc", bufs=8)
                nc.sync.dma_start(
                    out=poh[:],
                    in_=inp[f"pool_{br}"].ap()[:, t * Gpc : (t + 1) * Gpc],
                )
                nc.tensor.matmul(
                    out=ps[:], lhsT=u[:, t * H : (t + 1) * H], rhs=poh[:],
                    start=(t == 0), stop=(t == nb - 1),
                )
            nc.vector.tensor_tensor(
                out=hcT[row0 : row0 + H, :], in0=ps[:],
                in1=icrep[br][:H, :], op=OP.mult,
            )

        def head():
            fT_ps = ppool.tile([H, Gpc], F32, tag="ps_t", bufs=2)
            nc.tensor.matmul(
                out=fT_ps[:], lhsT=Wf1[:], rhs=hcT[:], start=True, stop=True
            )
            fT = spool.tile([H, Gpc], F32, tag="fT")
            nc.scalar.activation(
                out=fT[:], in_=fT_ps[:], func=ACT.Relu, bias=bf1T[:]
            )
            o_ps = ppool.tile([Gpc, 2], F32, tag="ps_u", bufs=2)
            nc.tensor.matmul(
                out=o_ps[:], lhsT=fT[:], rhs=Wf2[:], start=True, stop=True
            )
            o_sb = spool.tile([Gpc, 2], F32, tag="o_sb")
            nc.vector.tensor_tensor(
                out=o_sb[:], in0=o_ps[:], in1=bf2rep[:Gpc, :], op=OP.add
            )
            if probe_acc is not None:
                nc.vector.tensor_tensor(
                    out=o_sb[:], in0=o_sb[:],
                    in1=_ap(probe_acc, 0, [(1, Gpc), (0, 2)]), op=OP.add,
                )
            nc.sync.dma_start(out=out_t.ap(), in_=o_sb[:])

        def allgather(br, li):
            nc.gpsimd.collective_compute(
                "AllGather", OP.bypass, replica_groups=rg,
                ins=[part[li, br].ap()], outs=[table[li, br].ap()],
            )

        def allreduce(br, li):
            nc.gpsimd.collective_compute(
                "AllReduce", OP.add, replica_groups=rg,
                ins=[st_in[(br, li)].ap()], outs=[st_out[(br, li)].ap()],
            )

        def bn_chain(br, li):
            screp, shrep = bn_finish(br, li)
            bn_apply(br, screp, shrep)

        # =============================== flow ===============================
        # Emission order keeps the SWDGE gather queues busy: each branch's
        # stats/AllReduce/BN/table/AllGather chain is emitted inside the other
        # branch's aggregation (Pool sees the collective only after the other
        # branch's first-group desc-gens, so transfers cover the sync).
        if stop == "a":
            tt0 = spool.tile([Gpc, 2], F32, tag="tt0")
            nc.gpsimd.memset(tt0[:], 0.0)
            o = spool.tile([Gpc, 2], F32, tag="o_sb")
            nc.vector.tensor_copy(out=o[:], in_=tt0[:])
            nc.sync.dma_start(out=out_t.ap(), in_=o[:])
        elif _build.flow == "seq":
            build_table("c", 0)
            allgather("c", 0)
            build_table("s", 0)
            allgather("s", 0)
            for br in ("c", "s"):
                aggregate(br, 0)
                agg_finish(br, 0)
                layer_stats(br, 0)
                allreduce(br, 0)
            for br in ("c", "s"):
                bn_chain(br, 0)
                build_table(br, 1)
                allgather(br, 1)
            for br in ("c", "s"):
                aggregate(br, 1)
                agg_finish(br, 1)
                layer_stats(br, 1)
                allreduce(br, 1)
            for br in ("c", "s"):
                bn_chain(br, 1)
                pool_branch(br)
            head()
        else:
            build_table("c", 0)
            allgather("c", 0)
            build_table("s", 0)
            allgather("s", 0)
            ngc = len(cfg["c"]["groups"])
            ngs = len(cfg["s"]["groups"])
            aggregate("c", 0)
            agg_finish("c", 0)
            layer_stats("c", 0)
            aggregate("s", 0, [0])
            allreduce("c", 0)
            bn_chain("c", 0)
            build_table("c", 1)
            allgather("c", 1)
            aggregate("s", 0, range(1, ngs))
            agg_finish("s", 0)
            layer_stats("s", 0)
            aggregate("c", 1, [0])
            allreduce("s", 0)
            bn_chain("s", 0)
            build_table("s", 1)
            allgather("s", 1)
            aggregate("c", 1, range(1, ngc))
            agg_finish("c", 1)
            layer_stats("c", 1)
            aggregate("s", 1, [0])
            allreduce("c", 1)
            bn_chain("c", 1)
            pool_branch("c")
            aggregate("s", 1, range(1, ngs))
            agg_finish("s", 1)
            layer_stats("s", 1)
            allreduce("s", 1)
            bn_chain("s", 1)
            pool_branch("s")
            head()

        for p in (xpool, ppool, spool, ohpool, gxpool, gpool, upool, wpool,
                  consts):
            p.release()

    nc.compile()
    return nc


def kernel(_G=G_DEFAULT, _trace=False, _return_results=False, _stop=None, **inputs):
    cfg, in_maps = _prep(inputs, _G, NCORES)
    nc = _build(cfg, stop=_stop)
    res = run_bass_kernel_spmd(
        nc, in_maps, core_ids=list(range(NCORES)), trace=_trace
    )
    out = np.concatenate([res.results[k]["out"] for k in range(NCORES)], axis=0)
    if _return_results:
        return out, res
    return out

